# revision 2
# baseline (speedup 1.0000x reference)
"""V2: dma_gather block design. 4 streams, 1 descriptor/point, J=512/instr, 4 SWDGE queues.

Tables padded to 16B rows in DRAM. Blocks: L0-2 = 16 rows (256B), L3 = 32 rows (512B).
Per point: block idx (int16, wrapped-16 layout, replicated to 8 gpsimd groups) + intra-block
row e (f32, natural layout). Gather output [128, m, elem]: slot s=m*128+p = point (p, c=4i+m).
Extraction: DVE select-trees (4 or 5 rounds) with broadcast masks; sum 4 LODs; store.
"""
import sys
sys.path.insert(0, '/opt/trn_rl_repo')
import numpy as np

from concourse import bass, bacc, mybir, library_config
from concourse.bass_utils import run_bass_kernel_spmd

N = 4194304
NCORES = 8
NC = N // NCORES
C = NC // 128
LODS = [128, 256, 512, 1024]
FEAT = 3
J = 1024                     # points per dma_gather instruction
CM = min(1024, C)            # macro-chunk columns
NMACRO = C // CM
IPM = CM * 128 // J          # gather instrs per stream per macro
GI = 2                       # instrs per extraction group
NGRP = IPM // GI
ROWS = [16, 16, 16, 32]      # rows per block
ELEM = [64, 64, 64, 128]     # f32 per block
JC = J // 128                # gather-out columns per instr (4)
GC = GI * JC                 # columns per group (16)

_cached = {}


def _sel_rounds(l):
    return 4 if l < 3 else 5


def _build():
    if "nc" in _cached:
        return _cached["nc"]
    nc = bacc.Bacc("TRN2", target_bir_lowering=False, num_swdge_queues=4,
                   detect_race_conditions=False)
    pts = nc.dram_tensor("pts", [NC, 2], mybir.dt.float32, kind="ExternalInput")
    cbs = [nc.dram_tensor(f"cb{i}", [r * r, FEAT], mybir.dt.float32, kind="ExternalInput")
           for i, r in enumerate(LODS)]
    out = nc.dram_tensor("out", [NC, FEAT], mybir.dt.float32, kind="ExternalOutput")
    tabs = [nc.dram_tensor(f"tab{i}", [r * r, 4], mybir.dt.float32) for i, r in enumerate(LODS)]

    s_pts = nc.alloc_semaphore("s_pts")      # pts load per macro (16/macro)
    s_pin = nc.alloc_semaphore("s_pin")      # prep chunk in
    s_pad = nc.alloc_semaphore("s_pad")      # prep chunk padded
    s_pout = nc.alloc_semaphore("s_pout")    # prep chunk stored
    s_idx = nc.alloc_semaphore("s_idx")      # idx+fold done per macro (1/macro)
    s_rep = nc.alloc_semaphore("s_rep")      # replication DMAs (16 per dma)
    s_mz = nc.alloc_semaphore("s_mz")
    s_fold = nc.alloc_semaphore("s_fold")
    s_acc = nc.alloc_semaphore("s_acc")      # extraction groups done (1/group global)
    s_cmp = nc.alloc_semaphore("s_cmp")      # outm compacted per macro
    s_out = nc.alloc_semaphore("s_out")      # out stores (16/macro)
    gsem = [nc.alloc_semaphore(f"g{q}") for q in range(4)]

    pts_sb = nc.alloc_sbuf_tensor("pts_sb", [128, 2 * CM], mybir.dt.float32)
    fa = nc.alloc_sbuf_tensor("fa", [128, CM], mybir.dt.float32)
    fb = nc.alloc_sbuf_tensor("fb", [128, CM], mybir.dt.float32)
    fx = nc.alloc_sbuf_tensor("fx", [128, CM], mybir.dt.float32)
    fy = nc.alloc_sbuf_tensor("fy", [128, CM], mybir.dt.float32)
    fr = nc.alloc_sbuf_tensor("fr", [128, CM], mybir.dt.float32)
    wtmp = nc.alloc_sbuf_tensor("wtmp", [128, 8 * CM], mybir.dt.int16)
    wbuf = [nc.alloc_sbuf_tensor(f"wb{l}", [128, 8 * CM], mybir.dt.int16) for l in range(4)]
    gd = [nc.alloc_sbuf_tensor(f"gd{l}", [128, 2 * GI * JC * ELEM[l]], mybir.dt.float32)
          for l in range(4)]
    h1 = nc.alloc_sbuf_tensor("h1", [128, GC * 65], mybir.dt.float32)
    h2 = nc.alloc_sbuf_tensor("h2", [128, GC * 33], mybir.dt.float32)
    h3 = nc.alloc_sbuf_tensor("h3", [128, GC * 17], mybir.dt.float32)
    h4 = nc.alloc_sbuf_tensor("h4", [128, GC * 9], mybir.dt.float32)
    res4 = [nc.alloc_sbuf_tensor(f"res{l}", [128, GC * 5], mybir.dt.float32) for l in range(4)]
    bp = [[nc.alloc_sbuf_tensor(f"bp{l}_{k}", [128, CM], mybir.dt.uint8)
           for k in range(_sel_rounds(l))] for l in range(4)]
    u1 = nc.alloc_sbuf_tensor("u1", [128, GC * 4], mybir.dt.float32)
    u2 = nc.alloc_sbuf_tensor("u2", [128, GC * 4], mybir.dt.float32)
    outm4 = nc.alloc_sbuf_tensor("outm4", [128, 4 * CM], mybir.dt.float32)
    outm3 = nc.alloc_sbuf_tensor("outm3", [128, 3 * CM], mybir.dt.float32)
    warm = nc.alloc_sbuf_tensor("warm", [128, 16], mybir.dt.float32)

    # prep chunking: per LOD, X rows/partition processed in chunks of <=512
    PSTEP = min(512, (2 * CM) // 3)
    prep = []
    for l in range(4):
        X = LODS[l] * LODS[l] // 128
        done = 0
        while done < X:
            step = min(PSTEP, X - done)
            prep.append((l, done, step))
            done += step
    NPREP = len(prep)
    prep_in = pts_sb   # reuse: [128, 2*CM] f32 >= 3*512
    prep_out = wbuf[0][:].bitcast(mybir.dt.float32)  # [128, 8*CM/2] f32 >= 4*512

    def floor_block(v, dst_conv):
        """dst_conv <- floor(dst_conv-source...) pattern applied in-place below."""

    with nc.Block() as block:
        # ================= sync engine =================
        @block.sync
        def _(s):
            # table prep: load chunk, wait pad, store chunk
            for i, (l, off, step) in enumerate(prep):
                src = cbs[l][:].rearrange("(p x) f -> p (x f)", p=128)
                s.dma_start(out=prep_in[:, :3 * step],
                            in_=src[:, 3 * off:3 * (off + step)]).then_inc(s_pin, 16)
                s.wait_ge(s_pad, i + 1)
                dstv = tabs[l][:].rearrange("(p x) f -> p (x f)", p=128)
                s.dma_start(out=dstv[:, 4 * off:4 * (off + step)],
                            in_=prep_out[:, :4 * step]).then_inc(s_pout, 16)
                s.wait_ge(s_pout, 16 * (i + 1))
            # main loop: per macro: load pts, replicate folded idxs, store output
            pv = pts[:].rearrange("(p c) t -> p (c t)", p=128)
            o3 = out[:].rearrange("(p c) t -> p (c t)", p=128)
            for m in range(NMACRO):
                s.dma_start(out=pts_sb[:],
                            in_=pv[:, 2 * m * CM:2 * (m + 1) * CM]).then_inc(s_pts, 16)
                if m >= 1:
                    for l in range(4):
                        s.wait_ge(gsem[l], 16 * m * IPM)  # macro m-1 gathers done
                for l in range(4):
                    s.wait_ge(s_idx, 4 * m + l + 1)
                    for g in range(4):
                        s.dma_start(out=wbuf[l][32 * g:32 * (g + 1), :],
                                    in_=wtmp[0:32, :]).then_inc(s_rep, 16)
                # output store for macro m (after compact)
                s.wait_ge(s_cmp, m + 1)
                s.dma_start(out=o3[:, 3 * m * CM:3 * (m + 1) * CM],
                            in_=outm3[:]).then_inc(s_out, 16)
            s.wait_ge(s_out, 16 * NMACRO)

        # ================= vector engine =================
        @block.vector
        def _(v):
            v.memset(prep_out[:], 0.0)
            for l in range(4):
                v.memset(wbuf[l][0:32, :], 0)
            v.drain()
            # ---- table prep padding ----
            for i, (l, off, step) in enumerate(prep):
                v.wait_ge(s_pin, 16 * (i + 1))
                si = prep_in[:, :3 * step].rearrange("p (x f) -> p x f", f=3)
                so = prep_out[:, :4 * step].rearrange("p (x f) -> p x f", f=4)
                v.tensor_copy(out=so[:, :, 0:3], in_=si[:])
                v.drain().then_inc(s_pad, 1)
                if i + 1 < len(prep):
                    v.wait_ge(s_pout, 16 * (i + 1))  # dont overwrite prep bufs early

            v.wait_ge(s_pout, 16 * NPREP)   # last prep store done (wbuf[0] free)
            for m in range(NMACRO):
                # ---- index computation for macro m ----
                v.wait_ge(s_pts, 16 * (m + 1))

                xv = pts_sb[:].rearrange("p (c t) -> p c t", t=2)
                for l, res in enumerate(LODS):
                    for axis, dst in ((0, fx), (1, fy)):
                        i32 = fb[:].bitcast(mybir.dt.int32)
                        v.tensor_scalar_mul(out=dst[:], in0=xv[:, :, axis], scalar1=float(res - 1))
                        v.drain()
                        v.tensor_copy(out=i32, in_=dst[:])
                        v.drain()
                        v.tensor_copy(out=fa[:], in_=i32)
                        v.drain()
                        v.tensor_sub(out=fb[:], in0=fa[:], in1=dst[:])
                        v.drain()
                        v.tensor_scalar(out=fb[:], in0=fb[:], scalar1=0.0, scalar2=None,
                                        op0=mybir.AluOpType.is_gt)
                        v.drain()
                        v.tensor_sub(out=dst[:], in0=fa[:], in1=fb[:])
                        v.drain()
                    # r = xi + yi*res
                    v.scalar_tensor_tensor(out=fr[:], in0=fy[:], scalar=float(res), in1=fx[:],
                                           op0=mybir.AluOpType.mult, op1=mybir.AluOpType.add)
                    v.drain()
                    # i = floor(r / ROWS), e = r - ROWS*i
                    inv = 1.0 / ROWS[l]
                    i32 = fb[:].bitcast(mybir.dt.int32)
                    v.tensor_scalar_mul(out=fx[:], in0=fr[:], scalar1=inv)
                    v.drain()
                    v.tensor_copy(out=i32, in_=fx[:])
                    v.drain()
                    v.tensor_copy(out=fa[:], in_=i32)
                    v.drain()
                    v.tensor_sub(out=fb[:], in0=fa[:], in1=fx[:])
                    v.drain()
                    v.tensor_scalar(out=fb[:], in0=fb[:], scalar1=0.0, scalar2=None,
                                    op0=mybir.AluOpType.is_gt)
                    v.drain()
                    v.tensor_sub(out=fa[:], in0=fa[:], in1=fb[:])   # fa = block idx (f32)
                    v.drain()
                    v.scalar_tensor_tensor(out=fx[:], in0=fa[:], scalar=float(-ROWS[l]),
                                           in1=fr[:], op0=mybir.AluOpType.mult,
                                           op1=mybir.AluOpType.add)  # e = r - ROWS*i
                    v.drain()
                    # bit-planes of e (destructive), MSB first: bp[l][k] = bit k of e
                    ecur, enext = fx, fr
                    for k in reversed(range(_sel_rounds(l))):
                        v.tensor_scalar(out=fb[:], in0=ecur[:], scalar1=float(1 << k),
                                        scalar2=None, op0=mybir.AluOpType.is_ge)
                        v.drain()
                        v.tensor_copy(out=bp[l][k][:], in_=fb[:])
                        v.scalar_tensor_tensor(out=enext[:], in0=fb[:], scalar=float(-(1 << k)),
                                               in1=ecur[:], op0=mybir.AluOpType.mult,
                                               op1=mybir.AluOpType.add)
                        v.drain()
                        ecur, enext = enext, ecur
                    # ---- fold fa -> wbuf[l][0:16] (wrap-16 int16), via rot-16 shuffle ----
                    wv = wbuf[l][0:16, :].rearrange("r (c q) -> r c q", q=8)
                    for q in (0, 2, 4, 6):
                        v.tensor_copy(out=wv[:, :, q], in_=fa[16 * q:16 * (q + 1), :])
                    v.stream_shuffle(out=fb[:], in_=fa[:], mask=[(i + 16) % 32 for i in range(32)])
                    v.drain()
                    for q in (1, 3, 5, 7):
                        v.tensor_copy(out=wv[:, :, q], in_=fb[16 * (q - 1):16 * (q - 1) + 16, :])
                    v.drain()
                    # duplicate [0:16] -> [16:32] into wtmp (quadrant-internal)
                    if m > 0 or l > 0:
                        v.wait_ge(s_rep, 16 * 4 * (4 * m + l))  # wtmp consumed by sync
                    v.stream_shuffle(out=wtmp[0:32, :], in_=wbuf[l][0:32, :],
                                     mask=[i % 16 for i in range(32)])
                    v.drain().then_inc(s_idx, 1)

                # ---- extraction groups ----
                for grp in range(NGRP):
                    for l in range(4):
                        v.wait_ge(gsem[l], 16 * (m * IPM + (grp + 1) * GI))
                    half = (grp % 2)
                    for l in range(4):
                        E = ELEM[l]
                        src = gd[l][:, half * GI * JC * E:(half + 1) * GI * JC * E]
                        src = src.rearrange("p (g e) -> p g e", e=E)
                        width = E // 2
                        cur = src
                        nr = _sel_rounds(l)
                        for rnd in range(nr):
                            dsts = {64: h1, 32: h2, 16: h3, 8: h4}
                            dstt = res4[l] if width == 4 else dsts[width]
                            S = width + 1
                            dview = dstt[:, :GC * S].rearrange("p (g e) -> p g e", e=S)[:, :, :width]
                            mcol = bp[l][nr - 1 - rnd][:, grp * GC:(grp + 1) * GC]
                            mview = mcol.unsqueeze(-1).to_broadcast([128, GC, width])
                            v.select(out=dview, mask=mview,
                                     on_true=cur[:, :, width:2 * width],
                                     on_false=cur[:, :, 0:width], add_drain=True)
                            v.drain()
                            cur = dview
                            width //= 2
                    # sum 4 LODs: res are [128, GC, 5] padded, use [:, :, :4]
                    r4 = [res4[l][:].rearrange("p (g e) -> p g e", e=5)[:, :, :4] for l in range(4)]
                    v.tensor_add(out=u1[:], in0=r4[0], in1=r4[1])
                    v.tensor_add(out=u2[:], in0=r4[2], in1=r4[3])
                    v.drain()
                    ov = outm4[:, 4 * grp * GC:4 * (grp + 1) * GC]
                    v.tensor_add(out=ov, in0=u1[:], in1=u2[:])
                    v.drain().then_inc(s_acc, 1)
                # ---- compact 4 -> 3 and hand to sync ----
                o4 = outm4[:].rearrange("p (c f) -> p c f", f=4)
                o3v = outm3[:].rearrange("p (c f) -> p c f", f=3)
                if m >= 1:
                    v.wait_ge(s_out, 16 * m)
                v.tensor_copy(out=o3v[:], in_=o4[:, :, 0:3])
                v.drain().then_inc(s_cmp, 1)

        # ================= gpsimd engine =================
        @block.gpsimd
        def _(gp):
            gp.load_library(library_config.mlp)
            gp.memzero(warm[:]).then_inc(s_mz, 1)
            gp.wait_ge(s_mz, 1)
            tv = [tabs[l][:].rearrange("(b r) f -> b (r f)", r=ROWS[l]) for l in range(4)]
            for m in range(NMACRO):
                gp.wait_ge(s_rep, 16 * 16 * (m + 1))  # all 16 replication DMAs of macro m
                for grp in range(NGRP):
                    if m * NGRP + grp >= 2:
                        gp.wait_ge(s_acc, m * NGRP + grp - 1)  # gd half free
                    half = grp % 2
                    for i0 in range(GI):
                        i = grp * GI + i0
                        for l in range(4):
                            E = ELEM[l]
                            ndone = m * IPM + i   # instrs issued so far in queue l
                            if ndone >= 1:
                                gp.wait_ge(gsem[l], 16 * ndone)
                            dst = gd[l][:, (half * GI + i0) * JC * E:(half * GI + i0 + 1) * JC * E]
                            gp.dma_gather(
                                out_ap=dst.rearrange("p (c e) -> p c e", e=E),
                                in_ap=tv[l],
                                idxs_ap=wbuf[l][:, i * (J // 16):(i + 1) * (J // 16)],
                                num_idxs=J, num_idxs_reg=J, elem_size=E,
                                queue_num=l).then_inc(gsem[l], 16)
    nc.compile()
    _cached["nc"] = nc
    return nc


def _make_in_maps(inputs):
    pts = np.ascontiguousarray(inputs["pts"], dtype=np.float32)
    cbsv = [np.ascontiguousarray(inputs[f"cb{i}"], dtype=np.float32) for i in range(4)]
    in_maps = []
    for c in range(NCORES):
        in_maps.append({
            "pts": pts[c * NC:(c + 1) * NC],
            "cb0": cbsv[0], "cb1": cbsv[1], "cb2": cbsv[2], "cb3": cbsv[3],
        })
    return in_maps


def kernel(pts, cb0, cb1, cb2, cb3):
    nc = _build()
    in_maps = _make_in_maps(dict(pts=pts, cb0=cb0, cb1=cb1, cb2=cb2, cb3=cb3))
    res = run_bass_kernel_spmd(nc, in_maps, list(range(NCORES)))
    return np.concatenate([res.results[c]["out"] for c in range(NCORES)], axis=0)



# revision 3
# speedup vs baseline: 1.8954x; 1.8954x over previous
"""V2: dma_gather block design. 4 streams, 1 descriptor/point, J=512/instr, 4 SWDGE queues.

Tables padded to 16B rows in DRAM. Blocks: L0-2 = 16 rows (256B), L3 = 32 rows (512B).
Per point: block idx (int16, wrapped-16 layout, replicated to 8 gpsimd groups) + intra-block
row e (f32, natural layout). Gather output [128, m, elem]: slot s=m*128+p = point (p, c=4i+m).
Extraction: DVE select-trees (4 or 5 rounds) with broadcast masks; sum 4 LODs; store.
"""
import sys
sys.path.insert(0, '/opt/trn_rl_repo')
import numpy as np

from concourse import bass, bacc, mybir, library_config
from concourse.bass_utils import run_bass_kernel_spmd

N = 4194304
NCORES = 8
NC = N // NCORES
C = NC // 128
LODS = [128, 256, 512, 1024]
FEAT = 3
J = 1024                     # points per dma_gather instruction
CM = min(1024, C)            # macro-chunk columns
NMACRO = C // CM
IPM = CM * 128 // J          # gather instrs per stream per macro
GI = 2                       # instrs per extraction group
NGRP = IPM // GI
ROWS = [16, 16, 16, 32]      # rows per block
ELEM = [64, 64, 64, 128]     # f32 per block
JC = J // 128                # gather-out columns per instr (4)
GC = GI * JC                 # columns per group (16)

_cached = {}


def _sel_rounds(l):
    return 4 if l < 3 else 5


def _build():
    if "nc" in _cached:
        return _cached["nc"]
    nc = bacc.Bacc("TRN2", target_bir_lowering=False, num_swdge_queues=4,
                   detect_race_conditions=False)
    pts = nc.dram_tensor("pts", [NC, 2], mybir.dt.float32, kind="ExternalInput")
    cbs = [nc.dram_tensor(f"cb{i}", [r * r, FEAT], mybir.dt.float32, kind="ExternalInput")
           for i, r in enumerate(LODS)]
    out = nc.dram_tensor("out", [NC, FEAT], mybir.dt.float32, kind="ExternalOutput")
    tabs = [nc.dram_tensor(f"tab{i}", [r * r, 4], mybir.dt.float32) for i, r in enumerate(LODS)]

    s_pts = nc.alloc_semaphore("s_pts")      # pts load per macro (16/macro)
    s_pin = nc.alloc_semaphore("s_pin")      # prep chunk in
    s_pad = nc.alloc_semaphore("s_pad")      # prep chunk padded
    s_pout = nc.alloc_semaphore("s_pout")    # prep chunk stored
    s_idx = nc.alloc_semaphore("s_idx")      # idx+fold done per macro (1/macro)
    s_rep = nc.alloc_semaphore("s_rep")      # replication DMAs (16 per dma)
    s_mz = nc.alloc_semaphore("s_mz")
    s_fold = nc.alloc_semaphore("s_fold")
    s_acc = nc.alloc_semaphore("s_acc")      # extraction groups done (1/group global)
    s_cmp = nc.alloc_semaphore("s_cmp")      # outm compacted per macro
    s_out = nc.alloc_semaphore("s_out")      # out stores (16/macro)
    gsem = [nc.alloc_semaphore(f"g{q}") for q in range(4)]

    pts_sb = nc.alloc_sbuf_tensor("pts_sb", [128, 2 * CM], mybir.dt.float32)
    fa = nc.alloc_sbuf_tensor("fa", [128, CM], mybir.dt.float32)
    fb = nc.alloc_sbuf_tensor("fb", [128, CM], mybir.dt.float32)
    fx = nc.alloc_sbuf_tensor("fx", [128, CM], mybir.dt.float32)
    fy = nc.alloc_sbuf_tensor("fy", [128, CM], mybir.dt.float32)
    fr = nc.alloc_sbuf_tensor("fr", [128, CM], mybir.dt.float32)
    wtmp = nc.alloc_sbuf_tensor("wtmp", [128, 8 * CM], mybir.dt.int16)
    wbuf = [nc.alloc_sbuf_tensor(f"wb{l}", [128, 8 * CM], mybir.dt.int16) for l in range(4)]
    gd = [nc.alloc_sbuf_tensor(f"gd{l}", [128, 2 * GI * JC * ELEM[l]], mybir.dt.float32)
          for l in range(4)]
    h1 = nc.alloc_sbuf_tensor("h1", [128, GC * 65], mybir.dt.float32)
    h2 = nc.alloc_sbuf_tensor("h2", [128, GC * 33], mybir.dt.float32)
    h3 = nc.alloc_sbuf_tensor("h3", [128, GC * 17], mybir.dt.float32)
    h4 = nc.alloc_sbuf_tensor("h4", [128, GC * 9], mybir.dt.float32)
    res4 = [nc.alloc_sbuf_tensor(f"res{l}", [128, GC * 5], mybir.dt.float32) for l in range(4)]
    bp = [[nc.alloc_sbuf_tensor(f"bp{l}_{k}", [128, CM], mybir.dt.uint8)
           for k in range(_sel_rounds(l))] for l in range(4)]
    u1 = nc.alloc_sbuf_tensor("u1", [128, GC * 4], mybir.dt.float32)
    u2 = nc.alloc_sbuf_tensor("u2", [128, GC * 4], mybir.dt.float32)
    outm4 = nc.alloc_sbuf_tensor("outm4", [128, 4 * CM], mybir.dt.float32)
    outm3 = nc.alloc_sbuf_tensor("outm3", [128, 3 * CM], mybir.dt.float32)
    warm = nc.alloc_sbuf_tensor("warm", [128, 16], mybir.dt.float32)

    # prep chunking: per LOD, X rows/partition processed in chunks of <=512
    PSTEP = min(512, (2 * CM) // 3)
    prep = []
    for l in range(4):
        X = LODS[l] * LODS[l] // 128
        done = 0
        while done < X:
            step = min(PSTEP, X - done)
            prep.append((l, done, step))
            done += step
    NPREP = len(prep)
    prep_in = pts_sb   # reuse: [128, 2*CM] f32 >= 3*512
    prep_out = wbuf[0][:].bitcast(mybir.dt.float32)  # [128, 8*CM/2] f32 >= 4*512

    def floor_block(v, dst_conv):
        """dst_conv <- floor(dst_conv-source...) pattern applied in-place below."""

    with nc.Block() as block:
        # ================= sync engine =================
        @block.sync
        def _(s):
            # table prep: load chunk, wait pad, store chunk
            for i, (l, off, step) in enumerate(prep):
                src = cbs[l][:].rearrange("(p x) f -> p (x f)", p=128)
                s.dma_start(out=prep_in[:, :3 * step],
                            in_=src[:, 3 * off:3 * (off + step)]).then_inc(s_pin, 16)
                s.wait_ge(s_pad, i + 1)
                dstv = tabs[l][:].rearrange("(p x) f -> p (x f)", p=128)
                s.dma_start(out=dstv[:, 4 * off:4 * (off + step)],
                            in_=prep_out[:, :4 * step]).then_inc(s_pout, 16)
                s.wait_ge(s_pout, 16 * (i + 1))
            # main loop: per macro: load pts, replicate folded idxs, store output
            pv = pts[:].rearrange("(p c) t -> p (c t)", p=128)
            o3 = out[:].rearrange("(p c) t -> p (c t)", p=128)
            for m in range(NMACRO):
                s.dma_start(out=pts_sb[:],
                            in_=pv[:, 2 * m * CM:2 * (m + 1) * CM]).then_inc(s_pts, 16)
                if m >= 1:
                    for l in range(4):
                        s.wait_ge(gsem[l], 16 * m * IPM)  # macro m-1 gathers done
                for l in range(4):
                    s.wait_ge(s_idx, 4 * m + l + 1)
                    for g in range(4):
                        s.dma_start(out=wbuf[l][32 * g:32 * (g + 1), :],
                                    in_=wtmp[0:32, :]).then_inc(s_rep, 16)
                # output store for macro m (after compact)
                s.wait_ge(s_cmp, m + 1)
                s.dma_start(out=o3[:, 3 * m * CM:3 * (m + 1) * CM],
                            in_=outm3[:]).then_inc(s_out, 16)
            s.wait_ge(s_out, 16 * NMACRO)

        # ================= vector engine =================
        @block.vector
        def _(v):
            v.memset(prep_out[:], 0.0)
            for l in range(4):
                v.memset(wbuf[l][0:32, :], 0)
            v.drain()
            # ---- table prep padding ----
            for i, (l, off, step) in enumerate(prep):
                v.wait_ge(s_pin, 16 * (i + 1))
                si = prep_in[:, :3 * step].rearrange("p (x f) -> p x f", f=3)
                so = prep_out[:, :4 * step].rearrange("p (x f) -> p x f", f=4)
                v.tensor_copy(out=so[:, :, 0:3], in_=si[:])
                v.drain().then_inc(s_pad, 1)
                if i + 1 < len(prep):
                    v.wait_ge(s_pout, 16 * (i + 1))  # dont overwrite prep bufs early

            v.wait_ge(s_pout, 16 * NPREP)   # last prep store done (wbuf[0] free)
            for m in range(NMACRO):
                # ---- index computation for macro m ----
                v.wait_ge(s_pts, 16 * (m + 1))

                xv = pts_sb[:].rearrange("p (c t) -> p c t", t=2)
                for l, res in enumerate(LODS):
                    for axis, dst in ((0, fx), (1, fy)):
                        i32 = fb[:].bitcast(mybir.dt.int32)
                        v.tensor_scalar_mul(out=dst[:], in0=xv[:, :, axis], scalar1=float(res - 1))
                        v.drain()
                        v.tensor_copy(out=i32, in_=dst[:])
                        v.drain()
                        v.tensor_copy(out=fa[:], in_=i32)
                        v.drain()
                        v.tensor_sub(out=fb[:], in0=fa[:], in1=dst[:])
                        v.drain()
                        v.tensor_scalar(out=fb[:], in0=fb[:], scalar1=0.0, scalar2=None,
                                        op0=mybir.AluOpType.is_gt)
                        v.drain()
                        v.tensor_sub(out=dst[:], in0=fa[:], in1=fb[:])
                        v.drain()
                    # r = xi + yi*res
                    v.scalar_tensor_tensor(out=fr[:], in0=fy[:], scalar=float(res), in1=fx[:],
                                           op0=mybir.AluOpType.mult, op1=mybir.AluOpType.add)
                    v.drain()
                    # i = floor(r / ROWS), e = r - ROWS*i
                    inv = 1.0 / ROWS[l]
                    i32 = fb[:].bitcast(mybir.dt.int32)
                    v.tensor_scalar_mul(out=fx[:], in0=fr[:], scalar1=inv)
                    v.drain()
                    v.tensor_copy(out=i32, in_=fx[:])
                    v.drain()
                    v.tensor_copy(out=fa[:], in_=i32)
                    v.drain()
                    v.tensor_sub(out=fb[:], in0=fa[:], in1=fx[:])
                    v.drain()
                    v.tensor_scalar(out=fb[:], in0=fb[:], scalar1=0.0, scalar2=None,
                                    op0=mybir.AluOpType.is_gt)
                    v.drain()
                    v.tensor_sub(out=fa[:], in0=fa[:], in1=fb[:])   # fa = block idx (f32)
                    v.drain()
                    v.scalar_tensor_tensor(out=fx[:], in0=fa[:], scalar=float(-ROWS[l]),
                                           in1=fr[:], op0=mybir.AluOpType.mult,
                                           op1=mybir.AluOpType.add)  # e = r - ROWS*i
                    v.drain()
                    # bit-planes of e (destructive), MSB first: bp[l][k] = bit k of e
                    ecur, enext = fx, fr
                    for k in reversed(range(_sel_rounds(l))):
                        v.tensor_scalar(out=fb[:], in0=ecur[:], scalar1=float(1 << k),
                                        scalar2=None, op0=mybir.AluOpType.is_ge)
                        v.drain()
                        v.tensor_copy(out=bp[l][k][:], in_=fb[:])
                        v.scalar_tensor_tensor(out=enext[:], in0=fb[:], scalar=float(-(1 << k)),
                                               in1=ecur[:], op0=mybir.AluOpType.mult,
                                               op1=mybir.AluOpType.add)
                        v.drain()
                        ecur, enext = enext, ecur
                    # ---- fold fa -> wbuf[l][0:16] (wrap-16 int16), via rot-16 shuffle ----
                    wv = wbuf[l][0:16, :].rearrange("r (c q) -> r c q", q=8)
                    for q in (0, 2, 4, 6):
                        v.tensor_copy(out=wv[:, :, q], in_=fa[16 * q:16 * (q + 1), :])
                    v.stream_shuffle(out=fb[:], in_=fa[:], mask=[(i + 16) % 32 for i in range(32)])
                    v.drain()
                    for q in (1, 3, 5, 7):
                        v.tensor_copy(out=wv[:, :, q], in_=fb[16 * (q - 1):16 * (q - 1) + 16, :])
                    v.drain()
                    # duplicate [0:16] -> [16:32] into wtmp (quadrant-internal)
                    if m > 0 or l > 0:
                        v.wait_ge(s_rep, 16 * 4 * (4 * m + l))  # wtmp consumed by sync
                    v.stream_shuffle(out=wtmp[0:32, :], in_=wbuf[l][0:32, :],
                                     mask=[i % 16 for i in range(32)])
                    v.drain().then_inc(s_idx, 1)

                # ---- extraction groups: in-place predicated halving ----
                for grp in range(NGRP):
                    for l in range(4):
                        v.wait_ge(gsem[l], 16 * (m * IPM + (grp + 1) * GI))
                    half = (grp % 2)
                    for l in range(4):
                        E = ELEM[l]
                        src = gd[l][:, half * GI * JC * E:(half + 1) * GI * JC * E]
                        src = src.rearrange("p (g e) -> p g e", e=E)
                        width = E // 2
                        nr = _sel_rounds(l)
                        for rnd in range(nr):
                            mcol = bp[l][nr - 1 - rnd][:, grp * GC:(grp + 1) * GC]
                            mview = mcol.unsqueeze(-1).to_broadcast([128, GC, width])
                            v.copy_predicated(out=src[:, :, 0:width], mask=mview,
                                              data=src[:, :, width:2 * width])
                            v.drain()
                            width //= 2
                    # sum 4 LODs: selected rows sit at gd[l][..., 0:4]
                    r4 = [gd[l][:, half * GI * JC * ELEM[l]:(half + 1) * GI * JC * ELEM[l]]
                          .rearrange("p (g e) -> p g e", e=ELEM[l])[:, :, 0:4] for l in range(4)]
                    v.tensor_add(out=u1[:], in0=r4[0], in1=r4[1])
                    v.tensor_add(out=u2[:], in0=r4[2], in1=r4[3])
                    v.drain()
                    ov = outm4[:, 4 * grp * GC:4 * (grp + 1) * GC]
                    v.tensor_add(out=ov, in0=u1[:], in1=u2[:])
                    v.drain().then_inc(s_acc, 1)
                # ---- compact 4 -> 3 and hand to sync ----
                o4 = outm4[:].rearrange("p (c f) -> p c f", f=4)
                o3v = outm3[:].rearrange("p (c f) -> p c f", f=3)
                if m >= 1:
                    v.wait_ge(s_out, 16 * m)
                v.tensor_copy(out=o3v[:], in_=o4[:, :, 0:3])
                v.drain().then_inc(s_cmp, 1)

        # ================= gpsimd engine =================
        @block.gpsimd
        def _(gp):
            gp.load_library(library_config.mlp)
            gp.memzero(warm[:]).then_inc(s_mz, 1)
            gp.wait_ge(s_mz, 1)
            tv = [tabs[l][:].rearrange("(b r) f -> b (r f)", r=ROWS[l]) for l in range(4)]
            for m in range(NMACRO):
                gp.wait_ge(s_rep, 16 * 16 * (m + 1))  # all 16 replication DMAs of macro m
                for grp in range(NGRP):
                    if m * NGRP + grp >= 2:
                        gp.wait_ge(s_acc, m * NGRP + grp - 1)  # gd half free
                    half = grp % 2
                    for i0 in range(GI):
                        i = grp * GI + i0
                        for l in range(4):
                            E = ELEM[l]
                            ndone = m * IPM + i   # instrs issued so far in queue l
                            if ndone >= 1:
                                gp.wait_ge(gsem[l], 16 * ndone)
                            dst = gd[l][:, (half * GI + i0) * JC * E:(half * GI + i0 + 1) * JC * E]
                            gp.dma_gather(
                                out_ap=dst.rearrange("p (c e) -> p c e", e=E),
                                in_ap=tv[l],
                                idxs_ap=wbuf[l][:, i * (J // 16):(i + 1) * (J // 16)],
                                num_idxs=J, num_idxs_reg=J, elem_size=E,
                                queue_num=l).then_inc(gsem[l], 16)
    nc.compile()
    _cached["nc"] = nc
    return nc


def _make_in_maps(inputs):
    pts = np.ascontiguousarray(inputs["pts"], dtype=np.float32)
    cbsv = [np.ascontiguousarray(inputs[f"cb{i}"], dtype=np.float32) for i in range(4)]
    in_maps = []
    for c in range(NCORES):
        in_maps.append({
            "pts": pts[c * NC:(c + 1) * NC],
            "cb0": cbsv[0], "cb1": cbsv[1], "cb2": cbsv[2], "cb3": cbsv[3],
        })
    return in_maps


def kernel(pts, cb0, cb1, cb2, cb3):
    nc = _build()
    in_maps = _make_in_maps(dict(pts=pts, cb0=cb0, cb1=cb1, cb2=cb2, cb3=cb3))
    res = run_bass_kernel_spmd(nc, in_maps, list(range(NCORES)))
    return np.concatenate([res.results[c]["out"] for c in range(NCORES)], axis=0)

